# revision 5
# baseline (speedup 1.0000x reference)
"""Mixture-of-Experts (top-2 of 8 experts) Trainium2 kernel, 8 NeuronCores.

Strategy (expert-parallel, per sharding hint):
  - Router gate is tiny (T x 1024 @ 1024 x 8) and determines the sharding
    itself, so it is evaluated on the host (in float64) as part of the
    dispatch step; each token is sent to its top-2 experts.
  - Expert e's FFN runs entirely on core e: tokens routed to expert e are
    gathered, padded to a common capacity C, and the dense
    gelu(x @ w1 + b1) @ w2 FFN runs on that core in bf16 with fp32
    accumulation (TensorE native rate).
  - The combine step (scale by softmax weight, add b2, scatter-add over the
    two expert contributions per token) runs on the host.

Device layout per core (expert e = core id):
  xT  [8, 128, C]   bf16  gathered tokens, transposed: xT[ko,p,t] = x_t[ko*128+p]
  w1  [8, 128, 4096] bf16  w1[e] with D split into 8 partition chunks
  w2  [32, 128, 1024] bf16 w2[e] with F split into 32 partition chunks
  b1  [128, 32]     f32   b1[e] chunked per partition
  yT  [8, 128, C]   f32   (gelu(x@w1+b1) @ w2)^T, combine applied on host
"""

import math

import ml_dtypes
import numpy as np

N_CORES = 8
D = 1024
F = 4096
E = 8
TOP_K = 2
KO = D // 128   # 8 partition chunks of D
FO = F // 128   # 32 partition chunks of F
CT = 512        # token tile (matmul moving dim)

BF16 = ml_dtypes.bfloat16

# Cache of compiled Bass modules keyed by token capacity C.
_NC_CACHE: dict[int, object] = {}

# Most recent BassKernelResults — exposed for the test harness (profiling).
LAST_RESULTS = None


def _token_tiles(C):
    """Split capacity C into matmul-friendly tiles (<=CT wide each).

    The remainder tile is placed FIRST: the PE can start on it after a
    small fraction of the DMA traffic, and it warms the HAM clock gate
    cheaply while the rest of the weights stream in.
    """
    rem = C % CT
    tiles = []
    off = 0
    if rem:
        tiles.append((0, rem))
        off = rem
    while off < C:
        tiles.append((off, CT))
        off += CT
    return tiles


def _build(C):
    import concourse.mybir as mybir
    from concourse import bacc
    from concourse.tile import TileContext

    fp32 = mybir.dt.float32
    bf16 = mybir.dt.bfloat16

    nc = bacc.Bacc(
        "TRN2", target_bir_lowering=False, debug=False, num_devices=N_CORES
    )
    xT = nc.dram_tensor("xT", [KO, 128, C], bf16, kind="ExternalInput")
    w1 = nc.dram_tensor("w1", [KO, 128, F], bf16, kind="ExternalInput")
    w2 = nc.dram_tensor("w2", [FO, 128, D], bf16, kind="ExternalInput")
    b1 = nc.dram_tensor("b1", [128, FO], fp32, kind="ExternalInput")
    yT = nc.dram_tensor("yT", [KO, 128, C], fp32, kind="ExternalOutput")

    with TileContext(nc) as tc:
        with (
            tc.tile_pool(name="wpool", bufs=1) as wpool,
            tc.tile_pool(name="xpool", bufs=2) as xpool,
            tc.tile_pool(name="hpool", bufs=1) as hpool,
            tc.tile_pool(name="ypool", bufs=4) as ypool,
            tc.tile_pool(name="ph", bufs=2, space="PSUM") as phpool,
            tc.tile_pool(name="py", bufs=2, space="PSUM") as pypool,
        ):
            w1_sb = wpool.tile([128, KO, F], bf16)
            w2_sb = wpool.tile([128, FO, D], bf16)
            b1_sb = wpool.tile([128, FO], fp32)
            nc.sync.dma_start(b1_sb[:], b1[:])

            tiles = _token_tiles(C)

            # First token tile's activations go first so the PE can start
            # as soon as the first quarter of w1 lands.
            x_first = xpool.tile([128, KO, CT], bf16, tag="x_sb")
            for ko in range(KO):
                nc.sync.dma_start(
                    x_first[:, ko, : tiles[0][1]], xT[ko, :, : tiles[0][1]]
                )
            # w1 split into F-eighths: the f-loop consumes slice q after
            # only its 8 chunk DMAs (1 MB) instead of the full 8 MB.
            FQ = F // 8
            for q in range(8):
                for ko in range(KO):
                    nc.sync.dma_start(
                        w1_sb[:, ko, q * FQ : (q + 1) * FQ],
                        w1[ko, :, q * FQ : (q + 1) * FQ],
                    )
            # w2 is only needed once the first tile's gelu output exists
            # (~80us in) — issue after w1 so it doesn't steal HBM bandwidth.
            for fo in range(FO):
                nc.sync.dma_start(w2_sb[:, fo], w2[fo])

            for ti, (off, tw) in enumerate(tiles):
                if ti == 0:
                    x_sb = x_first
                else:
                    x_sb = xpool.tile([128, KO, CT], bf16, tag="x_sb")
                    for ko in range(KO):
                        nc.sync.dma_start(
                            x_sb[:, ko, :tw], xT[ko, :, off : off + tw]
                        )
                h_sb = hpool.tile([128, FO, CT], bf16)
                for fo in range(FO):
                    ph = phpool.tile([128, CT], fp32)
                    for ko in range(KO):
                        nc.tensor.matmul(
                            ph[:, :tw],
                            lhsT=w1_sb[:, ko, fo * 128 : (fo + 1) * 128],
                            rhs=x_sb[:, ko, :tw],
                            start=(ko == 0),
                            stop=(ko == KO - 1),
                        )
                    nc.scalar.activation(
                        h_sb[:, fo, :tw],
                        ph[:, :tw],
                        mybir.ActivationFunctionType.Gelu,
                        bias=b1_sb[:, fo : fo + 1],
                    )
                for do in range(KO):
                    py = pypool.tile([128, CT], fp32)
                    for fo in range(FO):
                        nc.tensor.matmul(
                            py[:, :tw],
                            lhsT=w2_sb[:, fo, do * 128 : (do + 1) * 128],
                            rhs=h_sb[:, fo, :tw],
                            start=(fo == 0),
                            stop=(fo == FO - 1),
                        )
                    y_sb = ypool.tile([128, CT], fp32)
                    nc.vector.tensor_copy(y_sb[:, :tw], py[:, :tw])
                    nc.sync.dma_start(yT[do, :, off : off + tw], y_sb[:, :tw])

    nc.compile()
    return nc


def kernel(x, gate_w, w1, b1, w2, b2):
    from concourse.bass_utils import run_bass_kernel_spmd

    global LAST_RESULTS

    x = np.asarray(x, dtype=np.float32)
    gate_w = np.asarray(gate_w, dtype=np.float32)
    w1 = np.asarray(w1, dtype=np.float32)
    b1 = np.asarray(b1, dtype=np.float32)
    w2 = np.asarray(w2, dtype=np.float32)
    b2 = np.asarray(b2, dtype=np.float32)

    B, S, Din = x.shape
    assert Din == D and gate_w.shape == (D, E)
    T = B * S
    xf = x.reshape(T, D)

    # ---- Host router (replicated gate): logits, top-2, softmax weights ----
    logits = xf.astype(np.float64) @ gate_w.astype(np.float64)  # [T, E]
    idx0 = np.argmax(logits, axis=1)
    rows = np.arange(T)
    v0 = logits[rows, idx0]
    l2 = logits.copy()
    l2[rows, idx0] = -np.inf
    idx1 = np.argmax(l2, axis=1)
    v1 = l2[rows, idx1]
    # softmax over the two top logits
    e1 = np.exp(v1 - v0)
    cw0 = 1.0 / (1.0 + e1)
    cw1 = e1 / (1.0 + e1)

    # ---- Dispatch: gather token ids per expert ----
    token_ids = []
    combine_w = []
    for e in range(E):
        sel0 = idx0 == e
        sel1 = idx1 == e
        ids = np.nonzero(sel0 | sel1)[0]
        w = np.where(sel0[ids], cw0[ids], cw1[ids])
        token_ids.append(ids)
        combine_w.append(w)

    max_n = max(len(ids) for ids in token_ids)
    C = max(128, max_n + (max_n & 1))  # even for bf16 row alignment

    if C not in _NC_CACHE:
        _NC_CACHE[C] = _build(C)
    nc = _NC_CACHE[C]

    # ---- Build per-core input maps ----
    in_maps = []
    for e in range(E):
        ids = token_ids[e]
        n_e = len(ids)
        xT = np.zeros((KO, 128, C), dtype=BF16)
        if n_e:
            # [n_e, D] -> [D, n_e] -> [KO, 128, n_e]
            xg = xf[ids].T.reshape(KO, 128, n_e)
            xT[:, :, :n_e] = xg.astype(BF16)
        in_maps.append(
            {
                "xT": xT,
                "w1": np.ascontiguousarray(
                    w1[e].reshape(KO, 128, F).astype(BF16)
                ),
                "w2": np.ascontiguousarray(
                    w2[e].reshape(FO, 128, D).astype(BF16)
                ),
                "b1": np.ascontiguousarray(b1[e].reshape(FO, 128).T),
            }
        )

    res = run_bass_kernel_spmd(nc, in_maps, core_ids=list(range(N_CORES)))
    LAST_RESULTS = res

    # ---- Combine on host: out[t] += cw * (y_e[t] + b2[e]) ----
    out = np.zeros((T, D), dtype=np.float32)
    for e in range(E):
        ids = token_ids[e]
        n_e = len(ids)
        if n_e == 0:
            continue
        y_t = res.results[e]["yT"].reshape(D, C)[:, :n_e].T  # [n_e, D]
        out[ids] += combine_w[e][:, None].astype(np.float32) * (y_t + b2[e])

    return out.reshape(B, S, D)


# revision 6
# speedup vs baseline: 1.0841x; 1.0841x over previous
"""Mixture-of-Experts (top-2 of 8 experts) Trainium2 kernel, 8 NeuronCores.

Strategy (expert-parallel, per sharding hint):
  - Router gate is tiny (T x 1024 @ 1024 x 8) and determines the sharding
    itself, so it is evaluated on the host (in float64) as part of the
    dispatch step; each token is sent to its top-2 experts.
  - Expert e's FFN runs entirely on core e: tokens routed to expert e are
    gathered, padded to a common capacity C, and the dense
    gelu(x @ w1 + b1) @ w2 FFN runs on that core in bf16 with fp32
    accumulation (TensorE native rate).
  - The combine step (scale by softmax weight, add b2, scatter-add over the
    two expert contributions per token) runs on the host.

Device layout per core (expert e = core id):
  xT  [8, 128, C]   bf16  gathered tokens, transposed: xT[ko,p,t] = x_t[ko*128+p]
  w1  [8, 128, 4096] bf16  w1[e] with D split into 8 partition chunks
  w2  [32, 128, 1024] bf16 w2[e] with F split into 32 partition chunks
  b1  [128, 32]     f32   b1[e] chunked per partition
  yT  [8, 128, C]   f32   (gelu(x@w1+b1) @ w2)^T, combine applied on host
"""

import math

import ml_dtypes
import numpy as np

N_CORES = 8
D = 1024
F = 4096
E = 8
TOP_K = 2
KO = D // 128   # 8 partition chunks of D
FO = F // 128   # 32 partition chunks of F
CT = 512        # token tile (matmul moving dim)

BF16 = ml_dtypes.bfloat16

# Cache of compiled Bass modules keyed by token capacity C.
_NC_CACHE: dict[int, object] = {}

# Most recent BassKernelResults — exposed for the test harness (profiling).
LAST_RESULTS = None


def _token_tiles(C):
    """Split capacity C into matmul-friendly tiles (<=CT wide each).

    Full CT-wide tiles first (aligned offsets — DMA rows stay aligned and
    the early pipeline is uniform); the remainder tile runs last.
    """
    tiles = []
    off = 0
    while C - off >= CT:
        tiles.append((off, CT))
        off += CT
    if off < C:
        tiles.append((off, C - off))
    return tiles


def _build(C):
    import concourse.mybir as mybir
    from concourse import bacc
    from concourse.tile import TileContext

    fp32 = mybir.dt.float32
    bf16 = mybir.dt.bfloat16

    nc = bacc.Bacc(
        "TRN2", target_bir_lowering=False, debug=False, num_devices=N_CORES
    )
    xT = nc.dram_tensor("xT", [KO, 128, C], bf16, kind="ExternalInput")
    w1 = nc.dram_tensor("w1", [KO, 128, F], bf16, kind="ExternalInput")
    w2 = nc.dram_tensor("w2", [FO, 128, D], bf16, kind="ExternalInput")
    b1 = nc.dram_tensor("b1", [128, FO], fp32, kind="ExternalInput")
    yT = nc.dram_tensor("yT", [KO, 128, C], fp32, kind="ExternalOutput")

    with TileContext(nc) as tc:
        with (
            tc.tile_pool(name="wpool", bufs=1) as wpool,
            tc.tile_pool(name="xpool", bufs=2) as xpool,
            tc.tile_pool(name="hpool", bufs=1) as hpool,
            tc.tile_pool(name="ypool", bufs=4) as ypool,
            tc.tile_pool(name="ph", bufs=2, space="PSUM") as phpool,
            tc.tile_pool(name="py", bufs=2, space="PSUM") as pypool,
        ):
            w1_sb = wpool.tile([128, KO, F], bf16)
            w2_sb = wpool.tile([128, FO, D], bf16)
            b1_sb = wpool.tile([128, FO], fp32)
            nc.sync.dma_start(b1_sb[:], b1[:])

            tiles = _token_tiles(C)

            # First token tile's activations go first so the PE can start
            # as soon as the first quarter of w1 lands.
            x_first = xpool.tile([128, KO, CT], bf16, tag="x_sb")
            for ko in range(KO):
                nc.sync.dma_start(
                    x_first[:, ko, : tiles[0][1]], xT[ko, :, : tiles[0][1]]
                )
            # w1 split into F-eighths: the f-loop consumes slice q after
            # only its 8 chunk DMAs (1 MB) instead of the full 8 MB.
            FQ = F // 8
            for q in range(8):
                for ko in range(KO):
                    nc.sync.dma_start(
                        w1_sb[:, ko, q * FQ : (q + 1) * FQ],
                        w1[ko, :, q * FQ : (q + 1) * FQ],
                    )
            # w2 is only needed once the first tile's gelu output exists
            # (~80us in) — issue after w1 so it doesn't steal HBM bandwidth.
            for fo in range(FO):
                nc.sync.dma_start(w2_sb[:, fo], w2[fo])

            for ti, (off, tw) in enumerate(tiles):
                if ti == 0:
                    x_sb = x_first
                else:
                    x_sb = xpool.tile([128, KO, CT], bf16, tag="x_sb")
                    for ko in range(KO):
                        nc.sync.dma_start(
                            x_sb[:, ko, :tw], xT[ko, :, off : off + tw]
                        )
                h_sb = hpool.tile([128, FO, CT], bf16)
                for fo in range(FO):
                    ph = phpool.tile([128, CT], fp32)
                    for ko in range(KO):
                        nc.tensor.matmul(
                            ph[:, :tw],
                            lhsT=w1_sb[:, ko, fo * 128 : (fo + 1) * 128],
                            rhs=x_sb[:, ko, :tw],
                            start=(ko == 0),
                            stop=(ko == KO - 1),
                        )
                    nc.scalar.activation(
                        h_sb[:, fo, :tw],
                        ph[:, :tw],
                        mybir.ActivationFunctionType.Gelu,
                        bias=b1_sb[:, fo : fo + 1],
                    )
                for do in range(KO):
                    py = pypool.tile([128, CT], fp32)
                    for fo in range(FO):
                        nc.tensor.matmul(
                            py[:, :tw],
                            lhsT=w2_sb[:, fo, do * 128 : (do + 1) * 128],
                            rhs=h_sb[:, fo, :tw],
                            start=(fo == 0),
                            stop=(fo == FO - 1),
                        )
                    y_sb = ypool.tile([128, CT], fp32)
                    nc.vector.tensor_copy(y_sb[:, :tw], py[:, :tw])
                    nc.sync.dma_start(yT[do, :, off : off + tw], y_sb[:, :tw])

    nc.compile()
    return nc


def kernel(x, gate_w, w1, b1, w2, b2):
    from concourse.bass_utils import run_bass_kernel_spmd

    global LAST_RESULTS

    x = np.asarray(x, dtype=np.float32)
    gate_w = np.asarray(gate_w, dtype=np.float32)
    w1 = np.asarray(w1, dtype=np.float32)
    b1 = np.asarray(b1, dtype=np.float32)
    w2 = np.asarray(w2, dtype=np.float32)
    b2 = np.asarray(b2, dtype=np.float32)

    B, S, Din = x.shape
    assert Din == D and gate_w.shape == (D, E)
    T = B * S
    xf = x.reshape(T, D)

    # ---- Host router (replicated gate): logits, top-2, softmax weights ----
    logits = xf.astype(np.float64) @ gate_w.astype(np.float64)  # [T, E]
    idx0 = np.argmax(logits, axis=1)
    rows = np.arange(T)
    v0 = logits[rows, idx0]
    l2 = logits.copy()
    l2[rows, idx0] = -np.inf
    idx1 = np.argmax(l2, axis=1)
    v1 = l2[rows, idx1]
    # softmax over the two top logits
    e1 = np.exp(v1 - v0)
    cw0 = 1.0 / (1.0 + e1)
    cw1 = e1 / (1.0 + e1)

    # ---- Dispatch: gather token ids per expert ----
    token_ids = []
    combine_w = []
    for e in range(E):
        sel0 = idx0 == e
        sel1 = idx1 == e
        ids = np.nonzero(sel0 | sel1)[0]
        w = np.where(sel0[ids], cw0[ids], cw1[ids])
        token_ids.append(ids)
        combine_w.append(w)

    max_n = max(len(ids) for ids in token_ids)
    C = max(128, max_n + (max_n & 1))  # even for bf16 row alignment

    if C not in _NC_CACHE:
        _NC_CACHE[C] = _build(C)
    nc = _NC_CACHE[C]

    # ---- Build per-core input maps ----
    in_maps = []
    for e in range(E):
        ids = token_ids[e]
        n_e = len(ids)
        xT = np.zeros((KO, 128, C), dtype=BF16)
        if n_e:
            # [n_e, D] -> [D, n_e] -> [KO, 128, n_e]
            xg = xf[ids].T.reshape(KO, 128, n_e)
            xT[:, :, :n_e] = xg.astype(BF16)
        in_maps.append(
            {
                "xT": xT,
                "w1": np.ascontiguousarray(
                    w1[e].reshape(KO, 128, F).astype(BF16)
                ),
                "w2": np.ascontiguousarray(
                    w2[e].reshape(FO, 128, D).astype(BF16)
                ),
                "b1": np.ascontiguousarray(b1[e].reshape(FO, 128).T),
            }
        )

    res = run_bass_kernel_spmd(nc, in_maps, core_ids=list(range(N_CORES)))
    LAST_RESULTS = res

    # ---- Combine on host: out[t] += cw * (y_e[t] + b2[e]) ----
    out = np.zeros((T, D), dtype=np.float32)
    for e in range(E):
        ids = token_ids[e]
        n_e = len(ids)
        if n_e == 0:
            continue
        y_t = res.results[e]["yT"].reshape(D, C)[:, :n_e].T  # [n_e, D]
        out[ids] += combine_w[e][:, None].astype(np.float32) * (y_t + b2[e])

    return out.reshape(B, S, D)


# revision 8
# speedup vs baseline: 1.0879x; 1.0035x over previous
"""Mixture-of-Experts (top-2 of 8 experts) Trainium2 kernel, 8 NeuronCores.

Strategy (expert-parallel, per sharding hint):
  - Router gate is tiny (T x 1024 @ 1024 x 8) and determines the sharding
    itself, so it is evaluated on the host (in float64) as part of the
    dispatch step; each token is sent to its top-2 experts.
  - Expert e's FFN runs entirely on core e: tokens routed to expert e are
    gathered, padded to a common capacity C, and the dense
    gelu(x @ w1 + b1) @ w2 FFN runs on that core in bf16 with fp32
    accumulation (TensorE native rate).
  - The combine step (scale by softmax weight, add b2, scatter-add over the
    two expert contributions per token) runs on the host.

Device layout per core (expert e = core id):
  xT  [8, 128, C]   bf16  gathered tokens, transposed: xT[ko,p,t] = x_t[ko*128+p]
  w1  [8, 128, 4096] bf16  w1[e] with D split into 8 partition chunks
  w2  [32, 128, 1024] bf16 w2[e] with F split into 32 partition chunks
  b1  [128, 32]     f32   b1[e] chunked per partition
  yT  [8, 128, C]   f32   (gelu(x@w1+b1) @ w2)^T, combine applied on host
"""

import math

import ml_dtypes
import numpy as np

N_CORES = 8
D = 1024
F = 4096
E = 8
TOP_K = 2
KO = D // 128   # 8 partition chunks of D
FO = F // 128   # 32 partition chunks of F
CT = 512        # token tile (matmul moving dim)

BF16 = ml_dtypes.bfloat16

# Cache of compiled Bass modules keyed by token capacity C.
_NC_CACHE: dict[int, object] = {}

# Most recent BassKernelResults — exposed for the test harness (profiling).
LAST_RESULTS = None


def _token_tiles(C):
    """Split capacity C into matmul-friendly tiles (<=CT wide each).

    Full CT-wide tiles first (aligned offsets — DMA rows stay aligned and
    the early pipeline is uniform); the remainder tile runs last.
    """
    tiles = []
    off = 0
    while C - off >= CT:
        tiles.append((off, CT))
        off += CT
    if off < C:
        tiles.append((off, C - off))
    return tiles


def _build(C):
    import concourse.mybir as mybir
    from concourse import bacc
    from concourse.tile import TileContext

    fp32 = mybir.dt.float32
    bf16 = mybir.dt.bfloat16

    nc = bacc.Bacc(
        "TRN2", target_bir_lowering=False, debug=False, num_devices=N_CORES
    )
    xT = nc.dram_tensor("xT", [KO, 128, C], bf16, kind="ExternalInput")
    w1 = nc.dram_tensor("w1", [KO, 128, F], bf16, kind="ExternalInput")
    w2 = nc.dram_tensor("w2", [FO, 128, D], bf16, kind="ExternalInput")
    b1 = nc.dram_tensor("b1", [128, FO], fp32, kind="ExternalInput")
    yT = nc.dram_tensor("yT", [KO, 128, C], fp32, kind="ExternalOutput")

    with TileContext(nc) as tc:
        with (
            tc.tile_pool(name="wpool", bufs=1) as wpool,
            tc.tile_pool(name="xpool", bufs=2) as xpool,
            tc.tile_pool(name="hpool", bufs=1) as hpool,
            tc.tile_pool(name="ypool", bufs=4) as ypool,
            tc.tile_pool(name="ph", bufs=3, space="PSUM") as phpool,
            tc.tile_pool(name="py", bufs=3, space="PSUM") as pypool,
        ):
            w1_sb = wpool.tile([128, KO, F], bf16)
            w2_sb = wpool.tile([128, FO, D], bf16)
            b1_sb = wpool.tile([128, FO], fp32)
            nc.sync.dma_start(b1_sb[:], b1[:])

            tiles = _token_tiles(C)

            # First token tile's activations go first so the PE can start
            # as soon as the first slice of w1 lands. Single multi-dim-AP
            # DMAs keep the issue latency off the critical path.
            x_first = xpool.tile([128, KO, CT], bf16, tag="x_sb")
            tw0 = tiles[0][1]
            nc.sync.dma_start(
                x_first[:, :, :tw0],
                xT[:, :, :tw0].rearrange("ko p c -> p ko c"),
            )
            # w1 split into F-eighths: the f-loop consumes slice q after a
            # single 1 MB DMA instead of the full 8 MB.
            FQ = F // 8
            for q in range(8):
                nc.sync.dma_start(
                    w1_sb[:, :, q * FQ : (q + 1) * FQ],
                    w1[:, :, q * FQ : (q + 1) * FQ].rearrange("ko p f -> p ko f"),
                )
            # w2 is only needed once the first tile's gelu output exists
            # (~80us in) — issue after w1 so it doesn't steal HBM bandwidth.
            for fb in range(0, FO, 8):
                nc.sync.dma_start(
                    w2_sb[:, fb : fb + 8, :],
                    w2[fb : fb + 8].rearrange("fo p d -> p fo d"),
                )

            for ti, (off, tw) in enumerate(tiles):
                if ti == 0:
                    x_sb = x_first
                else:
                    x_sb = xpool.tile([128, KO, CT], bf16, tag="x_sb")
                    nc.sync.dma_start(
                        x_sb[:, :, :tw],
                        xT[:, :, off : off + tw].rearrange("ko p c -> p ko c"),
                    )
                h_sb = hpool.tile([128, FO, CT], bf16)
                for fo in range(FO):
                    ph = phpool.tile([128, CT], fp32)
                    for ko in range(KO):
                        nc.tensor.matmul(
                            ph[:, :tw],
                            lhsT=w1_sb[:, ko, fo * 128 : (fo + 1) * 128],
                            rhs=x_sb[:, ko, :tw],
                            start=(ko == 0),
                            stop=(ko == KO - 1),
                        )
                    nc.scalar.activation(
                        h_sb[:, fo, :tw],
                        ph[:, :tw],
                        mybir.ActivationFunctionType.Gelu,
                        bias=b1_sb[:, fo : fo + 1],
                    )
                for do in range(KO):
                    py = pypool.tile([128, CT], fp32)
                    for fo in range(FO):
                        nc.tensor.matmul(
                            py[:, :tw],
                            lhsT=w2_sb[:, fo, do * 128 : (do + 1) * 128],
                            rhs=h_sb[:, fo, :tw],
                            start=(fo == 0),
                            stop=(fo == FO - 1),
                        )
                    y_sb = ypool.tile([128, CT], fp32)
                    nc.vector.tensor_copy(y_sb[:, :tw], py[:, :tw])
                    nc.sync.dma_start(yT[do, :, off : off + tw], y_sb[:, :tw])

    nc.compile()
    return nc


def kernel(x, gate_w, w1, b1, w2, b2):
    from concourse.bass_utils import run_bass_kernel_spmd

    global LAST_RESULTS

    x = np.asarray(x, dtype=np.float32)
    gate_w = np.asarray(gate_w, dtype=np.float32)
    w1 = np.asarray(w1, dtype=np.float32)
    b1 = np.asarray(b1, dtype=np.float32)
    w2 = np.asarray(w2, dtype=np.float32)
    b2 = np.asarray(b2, dtype=np.float32)

    B, S, Din = x.shape
    assert Din == D and gate_w.shape == (D, E)
    T = B * S
    xf = x.reshape(T, D)

    # ---- Host router (replicated gate): logits, top-2, softmax weights ----
    logits = xf.astype(np.float64) @ gate_w.astype(np.float64)  # [T, E]
    idx0 = np.argmax(logits, axis=1)
    rows = np.arange(T)
    v0 = logits[rows, idx0]
    l2 = logits.copy()
    l2[rows, idx0] = -np.inf
    idx1 = np.argmax(l2, axis=1)
    v1 = l2[rows, idx1]
    # softmax over the two top logits
    e1 = np.exp(v1 - v0)
    cw0 = 1.0 / (1.0 + e1)
    cw1 = e1 / (1.0 + e1)

    # ---- Dispatch: gather token ids per expert ----
    token_ids = []
    combine_w = []
    for e in range(E):
        sel0 = idx0 == e
        sel1 = idx1 == e
        ids = np.nonzero(sel0 | sel1)[0]
        w = np.where(sel0[ids], cw0[ids], cw1[ids])
        token_ids.append(ids)
        combine_w.append(w)

    max_n = max(len(ids) for ids in token_ids)
    C = max(128, max_n + (max_n & 1))  # even for bf16 row alignment

    if C not in _NC_CACHE:
        _NC_CACHE[C] = _build(C)
    nc = _NC_CACHE[C]

    # ---- Build per-core input maps ----
    in_maps = []
    for e in range(E):
        ids = token_ids[e]
        n_e = len(ids)
        xT = np.zeros((KO, 128, C), dtype=BF16)
        if n_e:
            # [n_e, D] -> [D, n_e] -> [KO, 128, n_e]
            xg = xf[ids].T.reshape(KO, 128, n_e)
            xT[:, :, :n_e] = xg.astype(BF16)
        in_maps.append(
            {
                "xT": xT,
                "w1": np.ascontiguousarray(
                    w1[e].reshape(KO, 128, F).astype(BF16)
                ),
                "w2": np.ascontiguousarray(
                    w2[e].reshape(FO, 128, D).astype(BF16)
                ),
                "b1": np.ascontiguousarray(b1[e].reshape(FO, 128).T),
            }
        )

    res = run_bass_kernel_spmd(nc, in_maps, core_ids=list(range(N_CORES)))
    LAST_RESULTS = res

    # ---- Combine on host: out[t] += cw * (y_e[t] + b2[e]) ----
    out = np.zeros((T, D), dtype=np.float32)
    for e in range(E):
        ids = token_ids[e]
        n_e = len(ids)
        if n_e == 0:
            continue
        y_t = res.results[e]["yT"].reshape(D, C)[:, :n_e].T  # [n_e, D]
        out[ids] += combine_w[e][:, None].astype(np.float32) * (y_t + b2[e])

    return out.reshape(B, S, D)


# revision 12
# speedup vs baseline: 1.1338x; 1.0422x over previous
"""Mixture-of-Experts (top-2 of 8 experts) Trainium2 kernel, 8 NeuronCores.

Strategy (expert-parallel, per sharding hint):
  - Router gate is tiny (T x 1024 @ 1024 x 8) and determines the sharding
    itself, so it is evaluated on the host (in float64) as part of the
    dispatch step; each token is sent to its top-2 experts.
  - Expert e's FFN runs entirely on core e: tokens routed to expert e are
    gathered, padded to a common capacity C, and the dense
    gelu(x @ w1 + b1) @ w2 FFN runs on that core in bf16 with fp32
    accumulation (TensorE native rate).
  - The combine step (scale by softmax weight, add b2, scatter-add over the
    two expert contributions per token) runs on the host.

Device layout per core (expert e = core id). DRAM layouts are chosen so
every staging DMA moves long contiguous runs per SBUF partition (8-16 KB),
keeping HWDGE descriptor-generation off the critical path:
  xt  [n_tiles, 128, KO, CT] bf16  token tiles: xt[t,p,ko,c] = x_tok[t*CT+c, ko*128+p]
  w1  [8, 128, KO, 512]      bf16  w1[e]: w1[q,p,ko,ff] = w1_e[ko*128+p, q*512+ff]
  w2  [4, 128, 8, 1024]      bf16  w2[e]: w2[fb,p,fi,d] = w2_e[(fb*8+fi)*128+p, d]
  b1  [128, 32]              f32   b1[e] chunked per partition
  yT  [8, 128, C]            f32   (gelu(x@w1+b1) @ w2)^T, combine on host
"""

import math

import ml_dtypes
import numpy as np

N_CORES = 8
D = 1024
F = 4096
E = 8
TOP_K = 2
KO = D // 128   # 8 partition chunks of D
FO = F // 128   # 32 partition chunks of F
CT = 512        # token tile (matmul moving dim)

BF16 = ml_dtypes.bfloat16

# Cache of compiled Bass modules keyed by token capacity C.
_NC_CACHE: dict[int, object] = {}

# Most recent BassKernelResults — exposed for the test harness (profiling).
LAST_RESULTS = None


def _token_tiles(C):
    """Split capacity C into matmul-friendly tiles (<=CT wide each).

    Full CT-wide tiles first (aligned offsets — DMA rows stay aligned and
    the early pipeline is uniform); the remainder tile runs last.
    """
    tiles = []
    off = 0
    while C - off >= CT:
        tiles.append((off, CT))
        off += CT
    if off < C:
        tiles.append((off, C - off))
    return tiles


def _build(C):
    import concourse.mybir as mybir
    from concourse import bacc
    from concourse.tile import TileContext

    fp32 = mybir.dt.float32
    bf16 = mybir.dt.bfloat16

    nc = bacc.Bacc(
        "TRN2", target_bir_lowering=False, debug=False, num_devices=N_CORES
    )
    tiles = _token_tiles(C)
    n_tiles = len(tiles)
    FQ = F // 8  # 512-wide F slice per w1 staging DMA
    xt = nc.dram_tensor("xt", [n_tiles, 128, KO, CT], bf16, kind="ExternalInput")
    w1 = nc.dram_tensor("w1", [8, 128, KO, FQ], bf16, kind="ExternalInput")
    w2 = nc.dram_tensor("w2", [4, 128, 8, D], bf16, kind="ExternalInput")
    b1 = nc.dram_tensor("b1", [128, FO], fp32, kind="ExternalInput")
    yT = nc.dram_tensor("yT", [KO, 128, C], fp32, kind="ExternalOutput")

    with TileContext(nc) as tc:
        with (
            tc.tile_pool(name="wpool", bufs=1) as wpool,
            tc.tile_pool(name="xpool", bufs=2) as xpool,
            tc.tile_pool(name="hpool", bufs=1) as hpool,
            tc.tile_pool(name="ypool", bufs=4) as ypool,
            tc.tile_pool(name="ph", bufs=3, space="PSUM") as phpool,
            tc.tile_pool(name="py", bufs=3, space="PSUM") as pypool,
        ):
            w1_sb = wpool.tile([128, 8, KO, FQ], bf16)
            w2_sb = wpool.tile([128, FO, D], bf16)
            b1_sb = wpool.tile([128, FO], fp32)
            nc.sync.dma_start(b1_sb[:], b1[:])

            # First token tile's activations go first so the PE can start
            # as soon as the first slice of w1 lands. Each staging DMA is a
            # single instruction whose per-partition runs are contiguous.
            x_first = xpool.tile([128, KO, CT], bf16, tag="x_sb")
            nc.sync.dma_start(x_first[:], xt[0])
            # w1 split into F-eighths: the f-loop consumes slice q after a
            # single 1 MB DMA instead of the full 8 MB.
            for q in range(8):
                nc.sync.dma_start(w1_sb[:, q], w1[q])
            # w2 is only needed once the first tile's gelu output exists
            # (~80us in) — issue after w1 so it doesn't steal HBM bandwidth.
            for fb in range(4):
                nc.sync.dma_start(w2_sb[:, fb * 8 : (fb + 1) * 8, :], w2[fb])

            for ti, (off, tw) in enumerate(tiles):
                if ti == 0:
                    x_sb = x_first
                else:
                    x_sb = xpool.tile([128, KO, CT], bf16, tag="x_sb")
                    nc.sync.dma_start(x_sb[:], xt[ti])
                h_sb = hpool.tile([128, FO, CT], bf16)
                for fo in range(FO):
                    q, fq = divmod(fo, 4)
                    ph = phpool.tile([128, CT], fp32)
                    for ko in range(KO):
                        nc.tensor.matmul(
                            ph[:, :tw],
                            lhsT=w1_sb[:, q, ko, fq * 128 : (fq + 1) * 128],
                            rhs=x_sb[:, ko, :tw],
                            start=(ko == 0),
                            stop=(ko == KO - 1),
                        )
                    nc.scalar.activation(
                        h_sb[:, fo, :tw],
                        ph[:, :tw],
                        mybir.ActivationFunctionType.Gelu,
                        bias=b1_sb[:, fo : fo + 1],
                    )
                for do in range(KO):
                    py = pypool.tile([128, CT], fp32)
                    for fo in range(FO):
                        nc.tensor.matmul(
                            py[:, :tw],
                            lhsT=w2_sb[:, fo, do * 128 : (do + 1) * 128],
                            rhs=h_sb[:, fo, :tw],
                            start=(fo == 0),
                            stop=(fo == FO - 1),
                        )
                    y_sb = ypool.tile([128, CT], fp32)
                    nc.vector.tensor_copy(y_sb[:, :tw], py[:, :tw])
                    nc.sync.dma_start(yT[do, :, off : off + tw], y_sb[:, :tw])

    nc.compile()
    return nc


def kernel(x, gate_w, w1, b1, w2, b2):
    from concourse.bass_utils import run_bass_kernel_spmd

    global LAST_RESULTS

    x = np.asarray(x, dtype=np.float32)
    gate_w = np.asarray(gate_w, dtype=np.float32)
    w1 = np.asarray(w1, dtype=np.float32)
    b1 = np.asarray(b1, dtype=np.float32)
    w2 = np.asarray(w2, dtype=np.float32)
    b2 = np.asarray(b2, dtype=np.float32)

    B, S, Din = x.shape
    assert Din == D and gate_w.shape == (D, E)
    T = B * S
    xf = x.reshape(T, D)

    # ---- Host router (replicated gate): logits, top-2, softmax weights ----
    logits = xf.astype(np.float64) @ gate_w.astype(np.float64)  # [T, E]
    idx0 = np.argmax(logits, axis=1)
    rows = np.arange(T)
    v0 = logits[rows, idx0]
    l2 = logits.copy()
    l2[rows, idx0] = -np.inf
    idx1 = np.argmax(l2, axis=1)
    v1 = l2[rows, idx1]
    # softmax over the two top logits
    e1 = np.exp(v1 - v0)
    cw0 = 1.0 / (1.0 + e1)
    cw1 = e1 / (1.0 + e1)

    # ---- Dispatch: gather token ids per expert ----
    token_ids = []
    combine_w = []
    for e in range(E):
        sel0 = idx0 == e
        sel1 = idx1 == e
        ids = np.nonzero(sel0 | sel1)[0]
        w = np.where(sel0[ids], cw0[ids], cw1[ids])
        token_ids.append(ids)
        combine_w.append(w)

    max_n = max(len(ids) for ids in token_ids)
    C = max(128, max_n + (max_n & 1))  # even for bf16 row alignment

    if C not in _NC_CACHE:
        _NC_CACHE[C] = _build(C)
    nc = _NC_CACHE[C]

    # ---- Build per-core input maps (layouts per module docstring) ----
    tiles = _token_tiles(C)
    n_tiles = len(tiles)
    in_maps = []
    for e in range(E):
        ids = token_ids[e]
        n_e = len(ids)
        xt = np.zeros((n_tiles, 128, KO, CT), dtype=BF16)
        if n_e:
            xg = xf[ids].astype(BF16)  # [n_e, D]
            for t, (off, tw) in enumerate(tiles):
                w = min(tw, max(0, n_e - off))
                if w <= 0:
                    break
                # [w, KO, 128] -> [128, KO, w]
                blk = xg[off : off + w].reshape(w, KO, 128).transpose(2, 1, 0)
                xt[t, :, :, :w] = blk
        # w1[e]: [D, F] -> [q, p, ko, ff]
        w1r = (
            w1[e]
            .reshape(KO, 128, 8, F // 8)
            .transpose(2, 1, 0, 3)
            .astype(BF16)
        )
        # w2[e]: [F, D] -> [fb, p, fi, d]
        w2r = (
            w2[e]
            .reshape(4, 8, 128, D)
            .transpose(0, 2, 1, 3)
            .astype(BF16)
        )
        in_maps.append(
            {
                "xt": np.ascontiguousarray(xt),
                "w1": np.ascontiguousarray(w1r),
                "w2": np.ascontiguousarray(w2r),
                "b1": np.ascontiguousarray(b1[e].reshape(FO, 128).T),
            }
        )

    res = run_bass_kernel_spmd(nc, in_maps, core_ids=list(range(N_CORES)))
    LAST_RESULTS = res

    # ---- Combine on host: out[t] += cw * (y_e[t] + b2[e]) ----
    out = np.zeros((T, D), dtype=np.float32)
    for e in range(E):
        ids = token_ids[e]
        n_e = len(ids)
        if n_e == 0:
            continue
        y_t = res.results[e]["yT"].reshape(D, C)[:, :n_e].T  # [n_e, D]
        out[ids] += combine_w[e][:, None].astype(np.float32) * (y_t + b2[e])

    return out.reshape(B, S, D)
